# revision 28
# baseline (speedup 1.0000x reference)
"""Trainium2 Bass kernel for nn_BiaffineSpan2WordLabeler.

Reference computation (B=4, L=128, IN=1024, H=512, NOUT=4):
    diff[b,i,j]  = x_const[b,j] - x_const[b,i]              # [B, L, L, IN]
    h1 = leaky(diff @ W1 + b1) * SCALE                      # [B, L*L, H]
    h2 = leaky(x_dep @ W2 + b2) * SCALE                     # [B, L, H]
    out[b,o,x,y] = sum_i h1b[b,x,i] Wa[o,i,j] h2[b,y,j]     # h1b = [h1, 1]

Algebraic restructurings (exact up to fp rounding):
  1. diff @ W1 = P[j] - P[i] where P = x_const @ W1; leaky applied after
     the elementwise assembly z[i,j] = P[j] - P[i] + b1.
  2. SCALE folded into W1,b1,W2,b2 host-side.
  3. Biaffine contracted as u[o,y,:] = Wa[o]·h2[y] first, then out = h1·u.

Sharding: 8 cores = (batch b = core//2) x (half of the i axis); SPMD via
host-side column permutation of x_const.

v4 reconstruction (best measured variant, 113.4us; costs from traces):
  * Matmul operands bf16 (same 1 row/cycle stream as fp32r, but LDWEIGHTS
    ~120ns vs 333ns so small matmuls pace ~2x faster; halves input DMA).
  * Output bf16 (halves the 16.8MB writeback that tail-dragged v1).
  * zg / pts / nsneg stay FP32: a bf16 broadcast tensor_tensor runs 3x
    slower on DVE (6.9us vs 2.1us per group, measured in v3).
  * u-stage as 16 big (512-row) matmuls in [y, h'] orientation, flipped
    into ucat[k][h', (o,y)] by PE transpose matmuls (bf16, 53ns) — XBAR
    DMA transposes cost ~1.2us each of queue time (v3 regression).
  * All 32 PSUM pair-drains on vector with the ubias add fused; h1 groups
    prefilled and the zg work split vector {0,4,8,11,14} / gpsimd (rest)
    so each group's h1 lands before the PE needs it.
  * Out-DMAs ride the sync queue only.
"""

import sys

_REPO = "/opt/trn_rl_repo"
if _REPO not in sys.path:
    sys.path.insert(0, _REPO)

import os as _os

import numpy as np

B, L, IND, HID, NOUT = 4, 128, 1024, 512, 4
SCALE = 1.0 / (HID**0.25)
NCORES = 8
ILOC = 64  # i-values per core
KH = 4  # HID / 128
CIN = 8  # IND / 128
G = 4  # i-values per leaky group
NG = ILOC // G
VEC_ZG = {0, 4, 8, 11, 14}  # zg groups on vector; rest on gpsimd
PREFILL = 5

_CACHED = {}


def _build_nc():
    import concourse.bass as bass
    import concourse.mybir as mybir
    from concourse.tile import TileContext
    import bass_rust

    F32 = mybir.dt.float32
    BF16 = mybir.dt.bfloat16
    AF = mybir.ActivationFunctionType
    ALU = mybir.AluOpType

    nc = bass.Bass()

    # [c*128+p, 0:512] = W1'[c*128+p, :], [c*128+p, 512:640] = xcT_perm[c*128+p, :]
    w1x = nc.dram_tensor("w1x", [IND, HID + L], BF16, kind="ExternalInput")
    w2x = nc.dram_tensor("w2x", [IND, HID + L], BF16, kind="ExternalInput")
    b1t = nc.dram_tensor("b1t", [128, KH], F32, kind="ExternalInput")
    b2t = nc.dram_tensor("b2t", [128, KH], F32, kind="ExternalInput")
    # wat[o][p, c*512 + i'] = Wa[o, i', c*128+p]   (i' < 512)
    wat = nc.dram_tensor("wat", [NOUT, 128, KH * HID], BF16, kind="ExternalInput")
    # wab[p, c*4+o] = Wa[o, 512, c*128+p]
    wab = nc.dram_tensor("wab", [128, KH * NOUT], BF16, kind="ExternalInput")
    idm = nc.dram_tensor("idm", [128, 128], BF16, kind="ExternalInput")
    out = nc.dram_tensor("out", [ILOC, L, NOUT * L], BF16, kind="ExternalOutput")

    with TileContext(nc) as tc:
        with (
            tc.tile_pool(name="constp", bufs=1) as constp,
            tc.tile_pool(name="wpool", bufs=3) as wpool,
            tc.tile_pool(name="watp", bufs=2) as watp,
            tc.tile_pool(name="pers", bufs=1) as pers,
            tc.tile_pool(name="work", bufs=4) as work,
            tc.tile_pool(name="h1pool", bufs=1) as h1pool,
            tc.tile_pool(name="outp", bufs=4) as outp,
            tc.tile_pool(name="ps1", bufs=2, space="PSUM") as ps1,
            tc.tile_pool(name="ps2", bufs=3, space="PSUM") as ps2,
        ):
            # ---- input DMAs: w1x/w2x on sync (critical path order), the
            # rest on scalar ----
            wx_all = [wpool.tile([128, HID + L], BF16, name="wx", tag="wx", bufs=CIN) for _ in range(CIN)]
            for c in range(CIN):
                nc.sync.dma_start(wx_all[c], w1x[c * 128 : (c + 1) * 128, :])
            w2_all = [wpool.tile([128, HID + L], BF16, name="wx", tag="wx", bufs=CIN) for _ in range(CIN)]
            for c in range(CIN):
                nc.sync.dma_start(w2_all[c], w2x[c * 128 : (c + 1) * 128, :])

            wato_all = [
                watp.tile([128, KH * HID], BF16, name="wato", bufs=NOUT)
                for _ in range(NOUT)
            ]
            for o in range(NOUT):
                nc.scalar.dma_start(wato_all[o], wat[o, :, :])

            b1t_sb = constp.tile([128, KH], F32)
            nc.scalar.dma_start(b1t_sb, b1t[:, :])
            b2t_sb = constp.tile([128, KH], F32)
            nc.scalar.dma_start(b2t_sb, b2t[:, :])
            wab_sb = constp.tile([128, KH * NOUT], BF16)
            nc.scalar.dma_start(wab_sb, wab[:, :])
            idm_sb = constp.tile([128, 128], BF16)
            nc.scalar.dma_start(idm_sb, idm[:, :])
            ones_f = constp.tile([1, 128], F32)
            nc.vector.memset(ones_f, 1.0)
            ones_r = constp.tile([1, 128], BF16)
            nc.vector.tensor_copy(ones_r, ones_f)

            # ---- persistent intermediates ----
            pts = pers.tile([128, KH * L], F32)  # P: [h', (k, j)]
            nsneg = pers.tile([128, KH * ILOC], F32)  # P[:,i] - b1: [h', (k,i)]
            h2t = pers.tile([128, KH * L], BF16)  # h2^T: [j', (c, y)]
            u2 = [pers.tile([128, HID], BF16, name=f"u2_{o}") for o in range(NOUT)]
            ucat = [
                pers.tile([128, NOUT * L], BF16, name=f"ucat{k}") for k in range(KH)
            ]
            ubias_r = pers.tile([1, NOUT * L], BF16)
            ubias_bc = pers.tile([128, NOUT * L], F32)

            # ---- stage P: pts[h'_k, j] = sum_in W1'[in, h'_k] xcT[in, j] ----
            for k in range(KH):
                pspt = ps1.tile([128, NOUT * L], F32, name="ps", tag="ps")
                for c in range(CIN):
                    nc.tensor.matmul(
                        pspt[:, 0:L],
                        wx_all[c][:, k * 128 : (k + 1) * 128],
                        wx_all[c][:, HID : HID + L],
                        start=(c == 0),
                        stop=(c == CIN - 1),
                    )
                nc.vector.tensor_copy(pts[:, k * L : (k + 1) * L], pspt[:, 0:L])

            pts_kj = pts.rearrange("p (k j) -> p k j", k=KH)
            nc.vector.tensor_tensor(
                nsneg.rearrange("p (k i) -> p k i", k=KH),
                pts_kj[:, :, 0:ILOC],
                b1t_sb[:, :, None].to_broadcast((128, KH, ILOC)),
                ALU.subtract,
            )
            nsneg_ki = nsneg.rearrange("p (k i) -> p k i", k=KH)

            # ---- stage h2: h2t[j'_k, y] = leaky(sum_in W2'[in,j'_k] xdT[in,y] + b2) ----
            for k in range(KH):
                psh2 = ps1.tile([128, NOUT * L], F32, name="ps", tag="ps")
                for c in range(CIN):
                    nc.tensor.matmul(
                        psh2[:, 0:L],
                        w2_all[c][:, k * 128 : (k + 1) * 128],
                        w2_all[c][:, HID : HID + L],
                        start=(c == 0),
                        stop=(c == CIN - 1),
                    )
                nc.scalar.activation(
                    h2t[:, k * L : (k + 1) * L],
                    psh2[:, 0:L],
                    AF.Prelu,
                    bias=b2t_sb[:, k : k + 1],
                    scale=1.0,
                    alpha=0.1,
                )

            # ---- stage u (transposed): u2[o][y,h'] = sum_j' h2[y,j'] Wa[o,h',j']
            # then PE-transpose into ucat[k][h'_k, (o,y)] ----
            def emit_psu(o):
                psu = ps1.tile([128, HID], F32, name="ps", tag="ps")
                for c in range(KH):
                    nc.tensor.matmul(
                        psu,
                        h2t[:, c * L : (c + 1) * L],
                        wato_all[o][:, c * HID : (c + 1) * HID],
                        start=(c == 0),
                        stop=(c == KH - 1),
                    )
                nc.scalar.activation(u2[o], psu, AF.Copy, bias=0.0, scale=1.0)

            def emit_tru(o):
                for k in range(KH):
                    tru = ps1.tile([128, 128], BF16, name="tru", tag="ps")
                    nc.tensor.transpose(tru, u2[o][:, k * 128 : (k + 1) * 128], idm_sb)
                    nc.scalar.activation(
                        ucat[k][:, o * L : (o + 1) * L], tru, AF.Copy,
                        bias=0.0, scale=1.0,
                    )

            emit_psu(0)
            emit_psu(1)
            emit_tru(0)
            emit_psu(2)
            emit_tru(1)
            emit_psu(3)
            emit_tru(2)

            # ---- stage ubias: ubias[(o,y)] = sum_j' Wa[o,512,j'] h2[y,j'] ----
            psub = ps1.tile([1, NOUT * L], F32, name="ps", tag="ps")
            for o in range(NOUT):
                for c in range(KH):
                    nc.tensor.matmul(
                        psub[0:1, o * L : (o + 1) * L],
                        wab_sb[:, c * NOUT + o : c * NOUT + o + 1],
                        h2t[:, c * L : (c + 1) * L],
                        start=(c == 0),
                        stop=(c == KH - 1),
                    )
            emit_tru(3)
            nc.vector.tensor_copy(ubias_r, psub)
            psbias = ps1.tile([128, NOUT * L], F32, name="ps", tag="ps")
            nc.tensor.matmul(psbias, ones_r, ubias_r, start=True, stop=True)
            nc.scalar.activation(ubias_bc, psbias, AF.Copy, bias=0.0, scale=1.0)

            # ---- h1 production + steady matmul/drain loop, interleaved ----
            h1_all = [
                h1pool.tile([128, G * KH * L], BF16, name=f"h1g{g}") for g in range(NG)
            ]

            def emit_zg(g):
                zg = work.tile([128, G * KH * L], F32, name="zg")
                zg_v = zg.rearrange("p (il k j) -> p il k j", il=G, k=KH)
                z_eng = nc.vector if g in VEC_ZG else nc.gpsimd
                z_eng.tensor_tensor(
                    zg_v,
                    pts_kj[:, None, :, :].to_broadcast((128, G, KH, L)),
                    nsneg_ki[:, :, g * G : (g + 1) * G]
                    .rearrange("p k i -> p i k")[:, :, :, None]
                    .to_broadcast((128, G, KH, L)),
                    ALU.subtract,
                )
                nc.scalar.activation(
                    h1_all[g], zg, AF.Prelu, bias=0.0, scale=1.0, alpha=0.1
                )

            for g in range(PREFILL):
                emit_zg(g)

            for g in range(NG):
                if g + PREFILL < NG:
                    emit_zg(g + PREFILL)
                h1g_v = h1_all[g].rearrange("p (il k j) -> p il k j", il=G, k=KH)
                for half in range(G // 2):
                    pso = ps2.tile([128, 2 * NOUT * L], F32, name="pso", tag="pso")
                    for sub in range(2):
                        il = half * 2 + sub
                        sl = pso[:, sub * NOUT * L : (sub + 1) * NOUT * L]
                        for k in range(KH):
                            nc.tensor.matmul(
                                sl,
                                h1g_v[:, il, k],
                                ucat[k],
                                start=(k == 0),
                                stop=(k == KH - 1),
                            )
                    osb = outp.tile([128, 2 * NOUT * L], BF16, name="osb")
                    nc.vector.tensor_tensor(
                        osb.rearrange("p (i f) -> p i f", i=2),
                        pso.rearrange("p (i f) -> p i f", i=2),
                        ubias_bc[:, None, :].to_broadcast((128, 2, NOUT * L)),
                        ALU.add,
                    )
                    pair = g * 2 + half
                    i0 = pair * 2
                    nc.sync.dma_start(
                        out[i0 : i0 + 2, :, :].rearrange("i p f -> p i f"),
                        osb.rearrange("p (i f) -> p i f", i=2),
                    )

    bass_rust.generate_event_semaphores(nc)
    return nc


def _prep_common(W1, b1, W2, b2, Wa):
    """Host-side weight preprocessing shared by all cores."""
    import ml_dtypes

    W1s = (np.asarray(W1, np.float32) * SCALE).astype(np.float32)
    b1s = (np.asarray(b1, np.float32) * SCALE).astype(np.float32)
    W2s = (np.asarray(W2, np.float32) * SCALE).astype(np.float32)
    b2s = (np.asarray(b2, np.float32) * SCALE).astype(np.float32)
    Wa = np.asarray(Wa, np.float32)

    b1t = np.ascontiguousarray(b1s.reshape(KH, 128).T)  # [128, KH]
    b2t = np.ascontiguousarray(b2s.reshape(KH, 128).T)

    # wat[o][p, c*512+i'] = Wa[o, i', c*128+p]
    watT = Wa.transpose(0, 2, 1)[:, :, :HID]  # [o, j, i']
    wat = np.ascontiguousarray(
        watT.reshape(NOUT, KH, 128, HID).transpose(0, 2, 1, 3).reshape(NOUT, 128, KH * HID)
    ).astype(ml_dtypes.bfloat16)
    # wab[p, c*4+o] = Wa[o, 512, c*128+p]
    wab = np.ascontiguousarray(
        Wa[:, HID, :].reshape(NOUT, KH, 128).transpose(2, 1, 0).reshape(128, KH * NOUT)
    ).astype(ml_dtypes.bfloat16)
    idm = np.eye(128, dtype=np.float32).astype(ml_dtypes.bfloat16)
    return W1s, W2s, b1t, b2t, wat, wab, idm


LAST_RESULT = None


def kernel(x_const, x_dep, W1, b1, W2, b2, Wa):
    global LAST_RESULT
    import ml_dtypes
    from concourse.bass_utils import run_bass_kernel_spmd

    x_const = np.asarray(x_const, np.float32)
    x_dep = np.asarray(x_dep, np.float32)
    W1s, W2s, b1t, b2t, wat, wab, idm = _prep_common(W1, b1, W2, b2, Wa)

    if "nc" not in _CACHED:
        _CACHED["nc"] = _build_nc()
    nc = _CACHED["nc"]

    in_maps = []
    perms = []
    for core in range(NCORES):
        b, ih = core // 2, core % 2
        perm = np.concatenate(
            [
                np.arange(ih * ILOC, (ih + 1) * ILOC),
                np.arange((1 - ih) * ILOC, (2 - ih) * ILOC),
            ]
        )
        perms.append(perm)
        xcT = np.ascontiguousarray(x_const[b].T[:, perm])  # [IND, L], cols permuted
        xdT = np.ascontiguousarray(x_dep[b].T)  # [IND, L]
        w1x = np.concatenate([W1s, xcT], axis=1).astype(ml_dtypes.bfloat16)
        w2x = np.concatenate([W2s, xdT], axis=1).astype(ml_dtypes.bfloat16)
        in_maps.append(
            {
                "w1x": w1x,
                "w2x": w2x,
                "b1t": b1t,
                "b2t": b2t,
                "wat": wat,
                "wab": wab,
                "idm": idm,
            }
        )

    _tdir = _os.environ.get("KERNEL_TRACE_DIR")
    _kw = {}
    if _tdir:
        _os.makedirs(_tdir, exist_ok=True)
        _kw["tmpdir"] = _tdir
    res = run_bass_kernel_spmd(nc, in_maps, core_ids=list(range(NCORES)), **_kw)
    LAST_RESULT = res

    out_full = np.empty((B, NOUT, L, L, L), np.float32)
    for core in range(NCORES):
        b, ih = core // 2, core % 2
        perm = perms[core]
        inv = np.argsort(perm)
        core_out = np.asarray(res.results[core]["out"]).astype(np.float32)
        core_out = core_out.reshape(ILOC, L, NOUT, L).transpose(2, 0, 1, 3)
        out_full[b, :, ih * ILOC : (ih + 1) * ILOC, :, :] = core_out[:, :, inv, :]
    return out_full
